# revision 4
# baseline (speedup 1.0000x reference)
"""Trainium2 Bass kernel for the AttFeatsCon contrastive loss.

reference:
    feats = l2norm(features)                       # [8192, 512]
    att   = l2norm(concat(noise, att_table[labels]))  # [8192, 600]
    dist  = exp(|feats@feats.T - att@att.T| / 0.05)
    pos   = sum(dist * same&~eye) / (n_pos + eps)
    neg   = sum(dist * ~same)     / (n_neg + eps)
    loss  = -log(pos / (pos + neg))

Strategy (8 NeuronCores, SPMD, no collectives):
  * X = [feats_hat | att_hat] with K = 512+600 = 1112; D = Xl @ Xr.T where
    the row-side operand sign-flips the att part, so one PSUM accumulation
    yields D = feats feats^T - att att^T.
  * dist is symmetric -> only compute each unordered block pair once.
    16 row-blocks of 512. Core c owns row-blocks c and c+8. Row-blocks
    0-7 process column-block offsets 0..8, row-blocks 8-15 offsets 0..7:
    every unordered pair covered exactly once, 17 col-blocks per core
    (identical instruction stream on every core; only data differs).
  * Per 512-column block: load raw rows, norms via ACT Square+accum,
    inv = rsqrt via DVE reciprocal + Newton (ACT Sqrt lives in a different
    activation-table set than Exp -> avoid table thrash), scale+round to
    float32r, PE-transpose into [K, 512] operand tiles.
  * Per 128-row subtile: 9 f32r matmuls (K-tiles) into one PSUM bank,
    |.| via any-engine abs_max, dist = ACT Exp(scale=20) with row-sum
    accumulator, same-class masked row-sum via one fused DVE
    scalar_tensor_tensor (labels equality x dist), eye extraction via the
    same trick with column-index iota on the two diagonal blocks.
  * Host: combines per-core partial sums (off-diag blocks weighted 2x),
    exact integer mask counts from labels, final -log on the reduced
    scalars.
"""

import os
import sys

for _p in ("/opt/trn_rl_repo", os.path.expanduser("~/.axon_site/_ro/trn_rl_repo")):
    if os.path.isdir(_p) and _p not in sys.path:
        sys.path.insert(0, _p)

import numpy as np

import concourse.bacc as bacc
import concourse.bass as bass
import concourse.tile as tile
from concourse import masks, mybir
from concourse.bass_utils import run_bass_kernel_spmd

F32 = mybir.dt.float32
F32R = mybir.dt.float32r
I32 = mybir.dt.int32
AF = mybir.ActivationFunctionType
OP = mybir.AluOpType

BS = 8192
FD = 512
AD = 300
CAT = AD + AD        # concat(noise, att) width = 600
KDIM = FD + CAT      # 1112
P = 128
B = 512              # block size
NB = BS // B         # 16 blocks
NCORES = 8
NIT = 17             # 9 col-blocks for row-block A, 8 for row-block B
TEMP = 0.05
ISCALE = 1.0 / TEMP  # 20.0
KT = [128] * 8 + [88]          # K-tile sizes (sum = 1112)
NKT = len(KT)

# accumulator slot layout
ACT_W = 72           # dist sums: slot it*4+s (68 used)
DVE_W = 80           # same sums: it*4+s (68 used); eye: 68+s (A), 72+s (B)
OUT_W = ACT_W + DVE_W

_module_cache = {}


def _build_module():
    nc = bacc.Bacc("TRN2", target_bir_lowering=False, debug=False,
                   num_devices=NCORES)

    rows_feat = nc.dram_tensor("rows_feat", [2, B, FD], F32, kind="ExternalInput")
    rows_noise = nc.dram_tensor("rows_noise", [2, B, AD], F32, kind="ExternalInput")
    cols_feat = nc.dram_tensor("cols_feat", [NB, B, FD], F32, kind="ExternalInput")
    cols_noise = nc.dram_tensor("cols_noise", [NB, B, AD], F32, kind="ExternalInput")
    att_t = nc.dram_tensor("att_t", [80, AD], F32, kind="ExternalInput")
    rows_lab_f = nc.dram_tensor("rows_lab_f", [P, 8], F32, kind="ExternalInput")
    rows_lab_i = nc.dram_tensor("rows_lab_i", [P, 8], I32, kind="ExternalInput")
    cols_lab_f = nc.dram_tensor("cols_lab_f", [NB, 1, B], F32, kind="ExternalInput")
    cols_lab_i = nc.dram_tensor("cols_lab_i", [NB, P, 4], I32, kind="ExternalInput")
    iota_b = nc.dram_tensor("iota_b", [1, B], F32, kind="ExternalInput")
    rowloc = nc.dram_tensor("rowloc", [P, 4], F32, kind="ExternalInput")

    acc_out = nc.dram_tensor("acc_out", [P, OUT_W], F32, kind="ExternalOutput")

    with tile.TileContext(nc) as tc:
        with (
            tc.tile_pool(name="consts", bufs=1) as consts,
            tc.tile_pool(name="lhs", bufs=1) as lhs_pool,
            tc.tile_pool(name="rhs", bufs=2) as rhs_pool,
            tc.tile_pool(name="raw", bufs=6) as raw_pool,
            tc.tile_pool(name="xr", bufs=6) as xr_pool,
            tc.tile_pool(name="sq", bufs=3) as sq_pool,
            tc.tile_pool(name="nrm", bufs=2) as nrm_pool,
            tc.tile_pool(name="ep", bufs=4) as ep_pool,
            tc.tile_pool(name="lab", bufs=2) as lab_pool,
            tc.tile_pool(name="tps", bufs=2, space="PSUM") as tp_ps,
            tc.tile_pool(name="mps", bufs=3, space="PSUM") as mm_ps,
        ):
            ident_f = consts.tile([P, P], F32)
            masks.make_identity(nc, ident_f[:])
            ident_r = consts.tile([P, P], F32R)
            nc.vector.tensor_copy(ident_r[:], ident_f[:])

            acc_act = consts.tile([P, ACT_W], F32)
            acc_dve = consts.tile([P, DVE_W], F32)
            nc.vector.memset(acc_act[:], 0.0)
            nc.vector.memset(acc_dve[:], 0.0)

            iota_bc = consts.tile([P, B], F32)
            nc.sync.dma_start(out=iota_bc[:], in_=iota_b.ap().broadcast_to((P, B)))
            rl_t = consts.tile([P, 4], F32)
            nc.sync.dma_start(out=rl_t[:], in_=rowloc[:, :])
            rlab_f = consts.tile([P, 8], F32)
            nc.sync.dma_start(out=rlab_f[:], in_=rows_lab_f[:, :])
            rlab_i = consts.tile([P, 8], I32)
            nc.sync.dma_start(out=rlab_i[:], in_=rows_lab_i[:, :])

            lhs = lhs_pool.tile([P, 2, NKT, B], F32R)

            def prep_512(feat_src, noise_src, gather_idx, dst, flip_att):
                """Normalize+round 512 rows into transposed [K, 512] tiles.

                feat_src(s)/noise_src(s): DRAM AP for rows s*128..s*128+128
                gather_idx(s): SBUF [P,1] int32 AP of labels for those rows
                dst(kt): SBUF AP [KT[kt], 512] float32r to fill
                flip_att: negate the att part (row-side operand)
                """
                n2 = nrm_pool.tile([P, 8], F32, tag="n2")
                raws = []
                for s in range(4):
                    raw = raw_pool.tile([P, KDIM], F32, tag="raw")
                    nc.sync.dma_start(out=raw[:, 0:FD], in_=feat_src(s))
                    nc.sync.dma_start(out=raw[:, FD:FD + AD], in_=noise_src(s))
                    nc.gpsimd.indirect_dma_start(
                        out=raw[:, FD + AD:KDIM], out_offset=None,
                        in_=att_t[:, :],
                        in_offset=bass.IndirectOffsetOnAxis(ap=gather_idx(s), axis=0))
                    raws.append(raw)
                    sqf = sq_pool.tile([P, FD], F32, tag="sqf")
                    nc.scalar.activation(sqf[:], raw[:, 0:FD], AF.Square,
                                         accum_out=n2[:, s:s + 1])
                    sqa = sq_pool.tile([P, CAT], F32, tag="sqa")
                    nc.scalar.activation(sqa[:], raw[:, FD:KDIM], AF.Square,
                                         accum_out=n2[:, 4 + s:5 + s])
                # inv = 1/sqrt(n2): reciprocal + Newton (seed 23*r covers
                # n2 in [350, 1100]; 5 iterations -> fp32-exact)
                r = nrm_pool.tile([P, 8], F32, tag="nr")
                nc.vector.reciprocal(r[:], n2[:])
                y = nrm_pool.tile([P, 8], F32, tag="ny")
                nc.vector.tensor_scalar(out=y[:], in0=r[:], scalar1=23.0,
                                        scalar2=None, op0=OP.mult)
                t = nrm_pool.tile([P, 8], F32, tag="nt")
                for _ in range(5):
                    nc.vector.tensor_tensor(out=t[:], in0=y[:], in1=y[:], op=OP.mult)
                    nc.vector.tensor_tensor(out=t[:], in0=t[:], in1=n2[:], op=OP.mult)
                    nc.vector.tensor_scalar(out=t[:], in0=t[:], scalar1=-0.5,
                                            scalar2=1.5, op0=OP.mult, op1=OP.add)
                    nc.vector.tensor_tensor(out=y[:], in0=y[:], in1=t[:], op=OP.mult)
                xrs = []
                for s in range(4):
                    xr = xr_pool.tile([P, NKT * P], F32R, tag="xr")
                    nc.vector.tensor_scalar(out=xr[:, 0:FD], in0=raws[s][:, 0:FD],
                                            scalar1=y[:, s:s + 1], scalar2=None,
                                            op0=OP.mult)
                    if flip_att:
                        nc.vector.tensor_scalar(out=xr[:, FD:KDIM],
                                                in0=raws[s][:, FD:KDIM],
                                                scalar1=y[:, 4 + s:5 + s],
                                                scalar2=-1.0,
                                                op0=OP.mult, op1=OP.mult)
                    else:
                        nc.vector.tensor_scalar(out=xr[:, FD:KDIM],
                                                in0=raws[s][:, FD:KDIM],
                                                scalar1=y[:, 4 + s:5 + s],
                                                scalar2=None, op0=OP.mult)
                    xrs.append(xr)
                for kt in range(NKT):
                    ks, kw = kt * P, KT[kt]
                    pt = tp_ps.tile([P, B], F32, tag="tps")
                    for s in range(4):
                        nc.tensor.transpose(pt[0:kw, s * P:(s + 1) * P].bitcast(F32R),
                                            xrs[s][:, ks:ks + kw], ident_r[:])
                    nc.vector.tensor_copy(dst(kt), pt[0:kw, :].bitcast(F32R))

            # ---- phase 0: row-side (lhsT) operands for blocks A and B ----
            for b in range(2):
                prep_512(
                    lambda s, b=b: rows_feat[b, s * P:(s + 1) * P, :],
                    lambda s, b=b: rows_noise[b, s * P:(s + 1) * P, :],
                    lambda s, b=b: rlab_i[:, 4 * b + s:4 * b + s + 1],
                    lambda kt, b=b: lhs[0:KT[kt], b, kt, :],
                    flip_att=True,
                )

            # ---- main loop: 17 (row-block, col-block) iterations ----
            for it in range(NIT):
                bsel = 0 if it < 9 else 1
                j = it if it < 9 else it - 1   # col slot in per-core order
                diag = (it == 0) or (it == 9)

                rhs = rhs_pool.tile([P, NKT, B], F32R, tag="rhs")
                clab_i = lab_pool.tile([P, 4], I32, tag="clabi")
                nc.sync.dma_start(out=clab_i[:], in_=cols_lab_i[j, :, :])
                clab_bc = lab_pool.tile([P, B], F32, tag="clabbc")
                nc.sync.dma_start(out=clab_bc[:],
                                  in_=cols_lab_f[j, :, :].broadcast_to((P, B)))
                prep_512(
                    lambda s, j=j: cols_feat[j, s * P:(s + 1) * P, :],
                    lambda s, j=j: cols_noise[j, s * P:(s + 1) * P, :],
                    lambda s, clab_i=clab_i: clab_i[:, s:s + 1],
                    lambda kt, rhs=rhs: rhs[0:KT[kt], kt, :],
                    flip_att=False,
                )

                for s in range(4):
                    pd = mm_ps.tile([P, B], F32, tag="mps")
                    for kt in range(NKT):
                        kw = KT[kt]
                        nc.tensor.matmul(pd[:, :],
                                         lhs[0:kw, bsel, kt, s * P:(s + 1) * P],
                                         rhs[0:kw, kt, :],
                                         start=(kt == 0), stop=(kt == NKT - 1))
                    absd = ep_pool.tile([P, B], F32, tag="absd")
                    nc.scalar.activation(absd[:], pd[:, :], AF.Abs, scale=ISCALE)
                    slot = it * 4 + s
                    dist = ep_pool.tile([P, B], F32, tag="dist")
                    nc.scalar.activation(dist[:], absd[:], AF.Exp,
                                         accum_out=acc_act[:, slot:slot + 1])
                    scr = ep_pool.tile([P, B], F32, tag="scr")
                    nc.vector.scalar_tensor_tensor(
                        out=scr[:], in0=clab_bc[:],
                        scalar=rlab_f[:, 4 * bsel + s:4 * bsel + s + 1],
                        in1=dist[:], op0=OP.is_equal, op1=OP.mult,
                        accum_out=acc_dve[:, slot:slot + 1])
                    if diag:
                        eslot = 68 + 4 * bsel + s
                        scr2 = ep_pool.tile([P, B], F32, tag="scr2")
                        nc.vector.scalar_tensor_tensor(
                            out=scr2[:], in0=iota_bc[:],
                            scalar=rl_t[:, s:s + 1],
                            in1=dist[:], op0=OP.is_equal, op1=OP.mult,
                            accum_out=acc_dve[:, eslot:eslot + 1])

            nc.sync.dma_start(out=acc_out[:, 0:ACT_W], in_=acc_act[:])
            nc.sync.dma_start(out=acc_out[:, ACT_W:OUT_W], in_=acc_dve[:])

    nc.finalize()
    return nc


def get_module():
    if "nc" not in _module_cache:
        _module_cache["nc"] = _build_module()
    return _module_cache["nc"]


def _host_prep(features, labels, att_table, noise):
    f = np.ascontiguousarray(features, dtype=np.float32).reshape(NB, B, FD)
    n = np.ascontiguousarray(noise, dtype=np.float32).reshape(NB, B, AD)
    lab = np.asarray(labels).reshape(NB, B)
    lab_f = lab.astype(np.float32)
    lab_i = lab.astype(np.int32)
    att = np.ascontiguousarray(att_table, dtype=np.float32)

    iota = np.arange(B, dtype=np.float32).reshape(1, B)
    rloc = np.arange(B, dtype=np.float32).reshape(4, P).T.copy()  # [P,4] col s

    in_maps = []
    for c in range(NCORES):
        perm = [(c + j) % NB for j in range(NB)]
        rsel = [c, c + 8]
        # [P, 8] col b*4+s = labels of rows s*128..(s+1)*128 of block rsel[b]
        rl = lab_f[rsel].reshape(2, 4, P).transpose(2, 0, 1).reshape(P, 8)
        ri = lab_i[rsel].reshape(2, 4, P).transpose(2, 0, 1).reshape(P, 8)
        ci = lab_i[perm].reshape(NB, 4, P).transpose(0, 2, 1)  # [NB, P, 4]
        in_maps.append({
            "rows_feat": np.ascontiguousarray(f[rsel]),
            "rows_noise": np.ascontiguousarray(n[rsel]),
            "cols_feat": np.ascontiguousarray(f[perm]),
            "cols_noise": np.ascontiguousarray(n[perm]),
            "att_t": att,
            "rows_lab_f": np.ascontiguousarray(rl),
            "rows_lab_i": np.ascontiguousarray(ri),
            "cols_lab_f": np.ascontiguousarray(lab_f[perm].reshape(NB, 1, B)),
            "cols_lab_i": np.ascontiguousarray(ci),
            "iota_b": iota,
            "rowloc": rloc,
        })
    return in_maps


def _combine(results, labels):
    s_dist_off = s_dist_diag = s_same_off = s_same_diag = s_eye = 0.0
    for r in results:
        a = r["acc_out"].astype(np.float64)
        act = a[:, 0:ACT_W]
        dve = a[:, ACT_W:OUT_W]
        for it in range(NIT):
            d = act[:, it * 4:it * 4 + 4].sum()
            sm = dve[:, it * 4:it * 4 + 4].sum()
            if it == 0 or it == 9:
                s_dist_diag += d
                s_same_diag += sm
            else:
                s_dist_off += d
                s_same_off += sm
        s_eye += dve[:, 68:76].sum()

    pos_num = 2.0 * s_same_off + s_same_diag - s_eye
    all_num = 2.0 * s_dist_off + s_dist_diag - s_eye
    neg_num = all_num - pos_num

    lab = np.asarray(labels).astype(np.int64)
    cnt = np.bincount(lab, minlength=80).astype(np.float64)
    same_tot = float((cnt * cnt).sum())
    n_pos = same_tot - BS
    n_neg = BS * BS - same_tot

    pos = pos_num / (n_pos + 1e-6)
    neg = neg_num / (n_neg + 1e-6)
    loss = -np.log(pos / (pos + neg))
    return np.float32(loss)


def kernel(features, labels, att_table, noise):
    nc = get_module()
    in_maps = _host_prep(features, labels, att_table, noise)
    res = run_bass_kernel_spmd(nc, in_maps, list(range(NCORES)))
    return _combine(res.results, labels)
